# revision 7
# baseline (speedup 1.0000x reference)
"""Trainium2 Bass kernel for dilated 3-tap per-channel softmax attention.

Reference computation (per batch):
    q = wq @ x                      # [O, T]
    xp = pad(x, d=4 both sides)     # [C, T+8]
    k = wk @ xp; v = wv @ xp        # [O, T+8]
    taps at offsets 0, 4, 8 (== t-4, t, t+4 in unpadded coords)
    scores = q * k_tap (per channel), softmax over the 3 taps,
    out = sum(attn * v_tap)         # [O, T]

Sharding: data-parallel over batch, 2 batches per core, 8 cores, no
collectives.

Key algebra: with D(j) = K(j) - K(j+4) over padded coords,
  s0 = q * (k0-k1) =  q * D(t),   s2 = q * (k2-k1) = -q * D(t+4)
D is computed directly on the PE as wk @ xd with xd(j) = xp(j) - xp(j+4)
pre-differenced on the host (fp16), so no K evacuation / on-chip
subtract is needed.  Softmax over {s0, 0, s2}:
  out = (r0*v0 + v1 + r2*v2) / (1 + r0 + r2),  r_i = exp(s_i)
Engine balance per tile-iteration (PE is the roofline at ~5.3us):
  PE  : D (fp16), Q, V (f32r) matmuls
  DVE : D evac (fp16), two score mults (fp16 2x), t0, reciprocal
  ACT : Q evac (fp16), V evac (bf16), exp(s0), exp(-s2raw)
  Pool: den (stt +1), t2, nt, num, ot   (bf16 outs only: f32 out is 2x)
"""

import sys
from contextlib import ExitStack

if "/opt/trn_rl_repo" not in sys.path:
    sys.path.insert(0, "/opt/trn_rl_repo")

import numpy as np

import concourse.bacc as bacc
import concourse.tile as tile
import concourse.mybir as mybir
from concourse.bass_utils import run_bass_kernel_spmd

B, C, T = 16, 512, 2048
O = 512
D = 4  # dilation == padding
TP = T + 2 * D  # padded time length, 2056
TD = T + D  # xd length, 2052
NCORES = 8
BPC = B // NCORES  # batches per core

KC = C // 128  # contraction chunks
MC = O // 128  # output-channel chunks
TT = 1024  # output cols per tile-iteration
XH = 1036  # first-half x/xd DMA split point

F32 = mybir.dt.float32
F32R = mybir.dt.float32r
BF16 = mybir.dt.bfloat16
FP16 = mybir.dt.float16
AF = mybir.ActivationFunctionType
ALU = mybir.AluOpType

_CACHED = {}


def _build(reps=1):
    nc = bacc.Bacc("TRN2", target_bir_lowering=False, debug=False)

    xp_d = [
        nc.dram_tensor(f"xp{b}", [C, TP], FP16, kind="ExternalInput").ap()
        for b in range(BPC)
    ]
    xd_d = [
        nc.dram_tensor(f"xd{b}", [C, TD], FP16, kind="ExternalInput").ap()
        for b in range(BPC)
    ]
    w_d = {
        "wqt": nc.dram_tensor("wqt", [C, O], FP16, kind="ExternalInput").ap(),
        "wvt": nc.dram_tensor("wvt", [C, O], FP16, kind="ExternalInput").ap(),
        "wkt": nc.dram_tensor("wkt", [C, O], FP16, kind="ExternalInput").ap(),
    }
    out_d = [
        nc.dram_tensor(f"out{b}", [O, T], BF16, kind="ExternalOutput").ap()
        for b in range(BPC)
    ]

    with tile.TileContext(nc) as tc, ExitStack() as ctx:
        wpool = ctx.enter_context(tc.tile_pool(name="w", bufs=1))
        xpool = ctx.enter_context(tc.tile_pool(name="x", bufs=2))
        sp2 = ctx.enter_context(tc.tile_pool(name="s2", bufs=2))
        sp3 = ctx.enter_context(tc.tile_pool(name="s3", bufs=3))
        opool = ctx.enter_context(tc.tile_pool(name="o", bufs=2))
        dpp = ctx.enter_context(tc.tile_pool(name="dp", bufs=1, space="PSUM"))
        qpp = ctx.enter_context(tc.tile_pool(name="qp", bufs=1, space="PSUM"))
        vpp = ctx.enter_context(tc.tile_pool(name="vp", bufs=1, space="PSUM"))

        # weights resident for the whole kernel: [kc][128, O] per projection.
        # wkt (needed by the first matmuls) loads first.
        wsb = {}
        for name, dt in (("wkt", FP16), ("wqt", FP16), ("wvt", FP16)):
            wsb[name] = [wpool.tile([128, O], dt, tag=f"{name}{kc}", name=f"{name}{kc}")
                         for kc in range(KC)]
        for kc in range(KC):
            nc.sync.dma_start(wsb["wkt"][kc][:], w_d["wkt"][kc * 128 : (kc + 1) * 128, :])

        def _one_pass(_iv=None):
          for b in range(BPC):
              # x (f32r, for Q/V) and xd (fp16, for D), DMA'd in two halves
              # so the first tile-iterations can start early.
              xsb, dsb = [], []
              for kc in range(KC):
                  dt_ = xpool.tile([128, TD], FP16, tag=f"xd{kc}", name=f"xd{kc}")
                  nc.sync.dma_start(
                      dt_[:, 0:XH], xd_d[b][kc * 128 : (kc + 1) * 128, 0:XH]
                  )
                  dsb.append(dt_)
              if b == 0 and _iv is None:
                  for kc in range(KC):
                      nc.sync.dma_start(
                          wsb["wqt"][kc][:], w_d["wqt"][kc * 128 : (kc + 1) * 128, :]
                      )
              for kc in range(KC):
                  xt = xpool.tile([128, TP], FP16, tag=f"x{kc}", name=f"x{kc}")
                  nc.sync.dma_start(
                      xt[:, 0:XH], xp_d[b][kc * 128 : (kc + 1) * 128, 0:XH]
                  )
                  xsb.append(xt)
              if b == 0 and _iv is None:
                  for kc in range(KC):
                      nc.sync.dma_start(
                          wsb["wvt"][kc][:], w_d["wvt"][kc * 128 : (kc + 1) * 128, :]
                      )
              for kc in range(KC):
                  nc.sync.dma_start(
                      dsb[kc][:, XH:TD], xd_d[b][kc * 128 : (kc + 1) * 128, XH:TD]
                  )
                  nc.sync.dma_start(
                      xsb[kc][:, XH:TP], xp_d[b][kc * 128 : (kc + 1) * 128, XH:TP]
                  )

              for th in range(0, T, TT):
                  for m in range(MC):
                      ms = slice(m * 128, (m + 1) * 128)
                      # ---- PE: D = wk @ xd over cols [th, th+TT+4) ----
                      dp = dpp.tile([128, TT + 4], F32, tag="dp", name="dp")
                      for kc in range(KC):
                          for n0, nn in ((0, 512), (512, 512), (1024, 4)):
                              nc.tensor.matmul(
                                  dp[:, n0 : n0 + nn],
                                  wsb["wkt"][kc][:, ms],
                                  dsb[kc][:, th + n0 : th + n0 + nn],
                                  start=(kc == 0),
                                  stop=(kc == KC - 1),
                              )
                      Dt = sp2.tile([128, TT + 4], FP16, tag="D", name="Dt")
                      nc.vector.tensor_copy(Dt[:], dp[:])

                      # ---- PE: Q over cols [th, th+TT) (padded offset +4) ----
                      qp = qpp.tile([128, TT], F32, tag="qp", name="qp")
                      for kc in range(KC):
                          for n0 in range(0, TT, 512):
                              nc.tensor.matmul(
                                  qp[:, n0 : n0 + 512],
                                  wsb["wqt"][kc][:, ms],
                                  xsb[kc][:, th + 4 + n0 : th + 4 + n0 + 512],
                                  start=(kc == 0),
                                  stop=(kc == KC - 1),
                              )
                      q16 = sp2.tile([128, TT], FP16, tag="q16", name="q16")
                      nc.scalar.activation(q16[:], qp[:], AF.Copy)

                      # ---- PE: V over padded cols [th, th+TT+8) ----
                      vp = vpp.tile([128, TT + 8], F32, tag="vp", name="vp")
                      for kc in range(KC):
                          for n0, nn in ((0, 512), (512, 512), (1024, 8)):
                              nc.tensor.matmul(
                                  vp[:, n0 : n0 + nn],
                                  wsb["wvt"][kc][:, ms],
                                  xsb[kc][:, th + n0 : th + n0 + nn],
                                  start=(kc == 0),
                                  stop=(kc == KC - 1),
                              )
                      vb = sp2.tile([128, TT + 8], BF16, tag="vb", name="vb")
                      for n0, nn in ((0, 512), (512, TT + 8 - 512)):
                          nc.scalar.activation(
                              vb[:, n0 : n0 + nn], vp[:, n0 : n0 + nn], AF.Copy
                          )

                      # scores (fp16): s0 = q*D[0:TT], s2raw = q*D[4:TT+4]
                      sb2 = sp2.tile([128, 2 * TT], FP16, tag="sboth", name="sb2")
                      nc.vector.tensor_tensor(
                          sb2[:, 0:TT], q16[:], Dt[:, 0:TT], ALU.mult
                      )
                      nc.vector.tensor_tensor(
                          sb2[:, TT : 2 * TT], q16[:], Dt[:, 4 : TT + 4], ALU.mult
                      )

                      # Post-score chain, in column blocks: full-width for
                      # steady state; the last two tile-iterations run in
                      # 512-col halves so the pipeline tail drains faster.
                      last2 = (th == T - TT) and (m >= MC - 2)
                      blocks = ((0, 512), (512, 512)) if last2 else ((0, TT),)
                      sfx = "h" if last2 else ""
                      rb = sp2.tile([128, 2 * TT], BF16, tag="rboth", name="rb")
                      for c0, cn in blocks:
                          cs = slice(c0, c0 + cn)
                          cs2 = slice(TT + c0, TT + c0 + cn)
                          # r0 = exp(s0), r2 = exp(-s2raw)  -> bf16
                          nc.scalar.activation(rb[:, cs], sb2[:, cs], AF.Exp)
                          nc.scalar.activation(
                              rb[:, cs2], sb2[:, cs2], AF.Exp, scale=-1.0
                          )
                          r0 = rb[:, cs]
                          r2 = rb[:, cs2]

                          # den = (r0 + r2) + 1 (bf16), inv = 1/den (f32, DVE)
                          # (scalar_tensor_tensor is not ISA-legal on Pool;
                          # plain tensor_scalar is)
                          den = sp3.tile([128, cn], BF16, tag=f"den{sfx}", name="den")
                          nc.gpsimd.tensor_tensor(den[:], r0, r2, ALU.add)
                          den2 = sp3.tile([128, cn], BF16, tag=f"den2{sfx}", name="den2")
                          nc.gpsimd.tensor_scalar_add(den2[:], den[:], 1.0)
                          inv = sp3.tile([128, cn], F32, tag=f"inv{sfx}", name="inv")
                          nc.vector.reciprocal(inv[:], den2[:])

                          # numerator: r0*v0 + v1 + r2*v2
                          t0 = sp3.tile([128, cn], BF16, tag=f"t0{sfx}", name="t0")
                          nc.vector.tensor_tensor(t0[:], r0, vb[:, c0 : c0 + cn], ALU.mult)
                          t2 = sp3.tile([128, cn], BF16, tag=f"t2{sfx}", name="t2")
                          nc.gpsimd.tensor_tensor(
                              t2[:], r2, vb[:, c0 + 8 : c0 + 8 + cn], ALU.mult
                          )
                          nt = sp3.tile([128, cn], BF16, tag=f"nt{sfx}", name="nt")
                          nc.vector.tensor_tensor(nt[:], t0[:], t2[:], ALU.add)
                          num = sp3.tile([128, cn], BF16, tag=f"num{sfx}", name="num")
                          nc.gpsimd.tensor_tensor(
                              num[:], nt[:], vb[:, c0 + 4 : c0 + 4 + cn], ALU.add
                          )

                          # out = num * inv (bf16; host upconverts to fp32)
                          ot = opool.tile([128, cn], BF16, tag=f"out{sfx}", name="ot")
                          nc.gpsimd.tensor_tensor(ot[:], num[:], inv[:], ALU.mult)
                          nc.sync.dma_start(
                              out_d[b][ms, th + c0 : th + c0 + cn], ot[:]
                          )

        if reps == 1:
            _one_pass()
        else:
            for kc in range(KC):
                nc.sync.dma_start(
                    wsb["wqt"][kc][:], w_d["wqt"][kc * 128 : (kc + 1) * 128, :]
                )
                nc.sync.dma_start(
                    wsb["wvt"][kc][:], w_d["wvt"][kc * 128 : (kc + 1) * 128, :]
                )
            with tc.For_i(0, reps, 1) as iv:
                _one_pass(iv)

    nc.finalize()
    return nc


def build_program(reps=1):
    # All ACT funcs used (Copy/Exp) live in one table set; restricting the
    # candidate list stops the table-load inserter (and the scheduler's cost
    # sim) from alternating table sets (~2.7us/load).  Must wrap the whole
    # build: the Tile scheduler simulates activation costs at TileContext
    # exit, before finalize.
    import concourse.hw_specs as _hw
    import concourse.bacc as _bacc_mod

    _orig = _hw.get_activation_tables
    _orig_bacc = _bacc_mod.get_activation_tables

    def _only_ln_exp(arch):
        t = _orig(arch)
        return {
            name: (fns if name == "natural_log_exp_and_others" else set())
            for name, fns in t.items()
        }

    _only_ln_exp.__wrapped_cache__ = {}

    def _cached_only_ln_exp(arch):
        if arch not in _only_ln_exp.__wrapped_cache__:
            _only_ln_exp.__wrapped_cache__[arch] = _only_ln_exp(arch)
        return _only_ln_exp.__wrapped_cache__[arch]

    _hw.get_activation_tables = _cached_only_ln_exp
    _bacc_mod.get_activation_tables = _cached_only_ln_exp
    try:
        return _build(reps)
    finally:
        _hw.get_activation_tables = _orig
        _bacc_mod.get_activation_tables = _orig_bacc


def _host_inputs(x, wq, wk, wv):
    xpad = np.zeros((B, C, TP), dtype=np.float32)
    xpad[:, :, D : D + T] = x
    xd = (xpad[:, :, :TD] - xpad[:, :, D:]).astype(np.float16)
    xpad = xpad.astype(np.float16)
    wqt = np.ascontiguousarray(wq.T).astype(np.float16)  # [C, O]
    wkt = np.ascontiguousarray(wk.T).astype(np.float16)
    wvt = np.ascontiguousarray(wv.T).astype(np.float16)
    in_maps = []
    for core in range(NCORES):
        m = {"wqt": wqt, "wkt": wkt, "wvt": wvt}
        for b in range(BPC):
            m[f"xp{b}"] = np.ascontiguousarray(xpad[core * BPC + b])
            m[f"xd{b}"] = np.ascontiguousarray(xd[core * BPC + b])
        in_maps.append(m)
    return in_maps


def kernel(x, wq, wk, wv):
    x = np.asarray(x)
    wq, wk, wv = np.asarray(wq), np.asarray(wk), np.asarray(wv)
    assert x.shape == (B, C, T) and x.dtype == np.float32
    if "nc" not in _CACHED:
        _CACHED["nc"] = build_program()
    nc = _CACHED["nc"]

    in_maps = _host_inputs(x, wq, wk, wv)

    trace = bool(_CACHED.get("trace"))
    res = run_bass_kernel_spmd(
        nc, in_maps, core_ids=list(range(NCORES)), trace=trace
    )
    if trace:
        _CACHED["last_exec_time_ns"] = res.exec_time_ns
        _CACHED["last_results"] = res
    out = np.empty((B, O, T), dtype=np.float32)
    for core in range(NCORES):
        for b in range(BPC):
            out[core * BPC + b] = res.results[core][f"out{b}"].astype(np.float32)
    return out
